# revision 25
# baseline (speedup 1.0000x reference)
"""DeepFM forward kernel for 8 Trainium2 NeuronCores (Bass/Tile), v5.

Single-phase design. Key structure (found via ntff profiling):

  - Data-parallel over batch: B=16384 -> 2048 rows/core; tables+weights
    replicated.
  - Emb lookup: per-(field, j-tile) transposed SWDGE gathers from
    [size_f, 128] bf16 tables -> feature-major [128, 512] tiles.  Rows
    are 256B with zero padding waste (v4's 512B stats-padded rows made
    the partition-strided RX transfers the critical path).
  - fc linear term: ONE whole-field indirect DMA per field (2048
    offsets in a single [128,16] offset AP) -> batch-major fcv; v3's 64
    chunked indirect DMAs paid 64x the ~1us SWDGE fixed cost.
  - FM: rowsum/rowsumsq via ones-vector matmuls on the PE (f32 PSUM
    accumulate over fields), squares on DVE; the global-scalar partial
    is AllGather'd across the 8 cores in-kernel (floor ~5us, hidden
    under the MLP) -- no second kernel launch (v3's phase B was 27us).
  - MLP in fp8 DoubleRow (2x via K=256/pass; the stream issues one
    512-col matmul every ~260ns when the HAM throttle is warm, so the
    emission keeps the PE stream gap-free: FM j -> L1 j -> L2 j-1 ...).
  - PSUM->SBUF activation drains split scalar/DVE per mt parity; input
    bf16->fp8 casts all on scalar (gpsimd cannot touch PSUM and its
    tensor ops are slow).
  - Tail: ypre ([1,2048], from L4) is DMA-transposed to batch-major
    [128,16], lin added, sigmoid with the AllGather'd S as bias.
"""

import os
import numpy as np
import ml_dtypes

# ---- problem constants (hardcoded; kernel.py must be self-contained) ----
TOTAL = 38279
CAT_SIZES = [31360, 6807, 18, 94]
EMB = 128
F = 4
B = 16384
N_CORES = 8
P = 128
NB = 512                       # matmul moving width (batch columns)
OFFSETS_NP = np.array([0, 31360, 38167, 38185], dtype=np.int32)

_build_cache = {}


def _build_main(b_loc, n_cores, use_cc):
    import concourse.bass as bass
    import concourse.mybir as mybir
    import concourse.tile as tile
    from concourse import bacc, library_config

    f32 = mybir.dt.float32
    bf16 = mybir.dt.bfloat16
    fp8 = mybir.dt.float8e4
    i16 = mybir.dt.int16
    i32 = mybir.dt.int32
    AF = mybir.ActivationFunctionType
    ALU = mybir.AluOpType
    AX = mybir.AxisListType
    DR = mybir.MatmulPerfMode.DoubleRow

    NJ = b_loc // NB             # 4 j-tiles
    NIX = NB // 16               # idx cols per (field, j) block
    NCH = b_loc // P             # 16 batch chunks of 128

    nc = bacc.Bacc(
        "TRN2",
        target_bir_lowering=False,
        debug=False,
        num_devices=n_cores,
    )

    # ---- DRAM I/O ----
    tabs = [
        nc.dram_tensor(f"tab{f}", [CAT_SIZES[f], EMB], bf16,
                       kind="ExternalInput").ap()
        for f in range(F)
    ]
    fc_d = nc.dram_tensor("fc", [TOTAL, 1], f32, kind="ExternalInput").ap()
    ix_d = nc.dram_tensor("ix", [P, NJ * F * NIX], i16,
                          kind="ExternalInput").ap()
    xig_d = nc.dram_tensor("xig", [P, F, NCH], i32, kind="ExternalInput").ap()
    w1q_d = nc.dram_tensor("w1q", [P, 4, 2048], fp8, kind="ExternalInput").ap()
    w2q_d = nc.dram_tensor("w2q", [P, 16, 1024], fp8, kind="ExternalInput").ap()
    w3q_d = nc.dram_tensor("w3q", [P, 8, 512], fp8, kind="ExternalInput").ap()
    w4q_d = nc.dram_tensor("w4q", [P, 4], fp8, kind="ExternalInput").ap()
    b1p_d = nc.dram_tensor("b1p", [P, 16], f32, kind="ExternalInput").ap()
    b2p_d = nc.dram_tensor("b2p", [P, 8], f32, kind="ExternalInput").ap()
    b3p_d = nc.dram_tensor("b3p", [P, 4], f32, kind="ExternalInput").ap()
    bc_d = nc.dram_tensor("bconst", [1, 1], f32, kind="ExternalInput").ap()
    if use_cc:
        y_d = nc.dram_tensor("y", [b_loc, 1], f32, kind="ExternalOutput").ap()
    else:
        ylin_d = nc.dram_tensor("ylin", [P, NCH], f32,
                                kind="ExternalOutput").ap()
        gpart_d = nc.dram_tensor("gpart", [1, 1], f32,
                                 kind="ExternalOutput").ap()

    with tile.TileContext(nc) as tc:
        with (
            tc.tile_pool(name="const", bufs=1) as const,
            tc.tile_pool(name="gat", bufs=1) as gat,
            tc.tile_pool(name="act", bufs=2) as actp,
            tc.tile_pool(name="psmm", bufs=2, space="PSUM") as psum_mm,
            tc.tile_pool(name="psfm", bufs=1, space="PSUM") as psum_fm,
            tc.tile_pool(name="psl4", bufs=2, space="PSUM") as psum_l4,
            tc.tile_pool(name="dram", bufs=1, space="DRAM") as dram,
        ):
            nc.gpsimd.load_library(library_config.mlp)

            # ---- small inputs first so the gathers can start early ----
            ix_sb = const.tile([P, NJ * F * NIX], i16, tag="ix_sb")
            nc.sync.dma_start(ix_sb[:], ix_d)
            xig = const.tile([P, F, NCH], i32, tag="xig")
            nc.sync.dma_start(xig[:], xig_d)
            bc_sb = const.tile([1, 1], f32, tag="bc_sb")
            nc.sync.dma_start(bc_sb[:], bc_d)
            b1p = const.tile([P, 16], f32, tag="b1p")
            nc.sync.dma_start(b1p[:], b1p_d)
            b2p = const.tile([P, 8], f32, tag="b2p")
            nc.sync.dma_start(b2p[:], b2p_d)
            b3p = const.tile([P, 4], f32, tag="b3p")
            nc.sync.dma_start(b3p[:], b3p_d)
            w4q = const.tile([P, 4], fp8, tag="w4q")
            nc.sync.dma_start(w4q[:], w4q_d)
            ones_col = const.tile([P, 1], bf16, tag="ones_col")
            nc.vector.memset(ones_col[:], 1.0)
            # big weights after the index tiles
            w1q = const.tile([P, 4, 2048], fp8, tag="w1q")
            nc.sync.dma_start(w1q[:], w1q_d)
            w2q = const.tile([P, 16, 1024], fp8, tag="w2q")
            nc.sync.dma_start(w2q[:], w2q_d)
            w3q = const.tile([P, 8, 512], fp8, tag="w3q")
            nc.sync.dma_start(w3q[:], w3q_d)

            ypre_sb = const.tile([1, b_loc], f32, tag="ypre_sb")
            ydram = dram.tile([1, b_loc], f32, tag="ydram")
            gacc = const.tile([1, NB], f32, tag="gacc")
            nc.vector.memset(gacc[:], 0.0)
            fcv = const.tile([P, F, NCH], f32, tag="fcv")

            def ixsl(f, j):
                k = (j * F + f) * NIX
                return ix_sb[:, k:k + NIX]

            # ---- gathers: j0 first (PE needs it), then fc, then j1-3 ----
            G = {}

            def emit_gathers(j):
                for f in range(F):
                    g = gat.tile([P, 1, NB], bf16, tag=f"g{f}_{j}",
                                 name=f"g{f}_{j}")
                    nc.gpsimd.dma_gather(
                        g[:], tabs[f], ixsl(f, j), NB, NB, EMB,
                        transpose=True, single_packet=False,
                    )
                    G[(f, j)] = g

            emit_gathers(0)
            for f in range(F):
                nc.gpsimd.indirect_dma_start(
                    out=fcv[:, f, :],
                    out_offset=None,
                    in_=fc_d,
                    in_offset=bass.IndirectOffsetOnAxis(ap=xig[:, f, :],
                                                        axis=0),
                )
            for j in range(1, NJ):
                emit_gathers(j)

            # fp8 pair tiles for L1 rhs: PT[g][:, c, :] = emb of field 2g+c
            PT = {}
            for j in range(NJ):
                for g in range(2):
                    PT[(g, j)] = gat.tile([P, 2, NB], fp8, tag=f"p{g}_{j}",
                                          name=f"p{g}_{j}")

            def emit_casts(j):
                for f in range(F):
                    nc.scalar.activation(PT[(f // 2, j)][:, f % 2, :],
                                         G[(f, j)][:, 0, :], AF.Copy)

            # ---- FM stats for one j-tile ----
            SQ = {}

            def emit_squares(j):
                for f in range(F):
                    sq = gat.tile([P, NB], bf16, tag=f"sq{f}_{j}",
                                  name=f"sq{f}_{j}")
                    nc.vector.tensor_tensor(out=sq[:], in0=G[(f, j)][:, 0, :],
                                            in1=G[(f, j)][:, 0, :],
                                            op=ALU.mult)
                    SQ[(f, j)] = sq

            def emit_fm_mm(j):
                psA = psum_fm.tile([1, NB], f32, tag="psA", name=f"psA{j}")
                for f in range(F):
                    nc.tensor.matmul(psA[:], lhsT=ones_col[:],
                                     rhs=G[(f, j)][:, 0, :],
                                     start=(f == 0), stop=(f == F - 1))
                psB = psum_fm.tile([1, NB], f32, tag="psB", name=f"psB{j}")
                for f in range(F):
                    nc.tensor.matmul(psB[:], lhsT=ones_col[:],
                                     rhs=SQ[(f, j)][:],
                                     start=(f == 0), stop=(f == F - 1))
                return psA, psB

            def emit_fm_tail(j, psA, psB):
                # NOTE: scalar AF.Square is table-based and inaccurate for
                # |x|~50 (rowsum range) — square on DVE instead
                rs = actp.tile([1, NB], f32, tag="fmr", name=f"fmr{j}")
                nc.scalar.activation(rs[:], psA[:], AF.Copy)
                t1 = actp.tile([1, NB], f32, tag="fmt", name=f"fmt{j}")
                nc.vector.tensor_tensor(out=t1[:], in0=rs[:], in1=rs[:],
                                        op=ALU.mult)
                nc.vector.tensor_tensor(out=t1[:], in0=t1[:], in1=psB[:],
                                        op=ALU.subtract)
                nc.vector.tensor_tensor(out=gacc[:], in0=gacc[:], in1=t1[:],
                                        op=ALU.add)

            def act_relu(on_scalar, dst, ps_slice, bias_ap):
                if on_scalar:
                    nc.scalar.activation(dst, ps_slice, AF.Relu, bias=bias_ap)
                else:
                    nc.vector.tensor_scalar(dst, ps_slice, bias_ap, 0.0,
                                            ALU.add, ALU.max)

            # ---- MLP layers for one j-tile ----
            H = {}

            def emit_layer(j, lno, KG, MT, wq, bp, rhs_of):
                Hout = [actp.tile([P, 2, NB], fp8, tag=f"h{lno}_{g}",
                                  name=f"h{lno}_{g}_{j}")
                        for g in range(MT // 2)]
                H[(lno, j)] = Hout
                for mt in range(MT):
                    q = mt % 2
                    if q == 0:
                        ps = psum_mm.tile([P, 2, NB], f32, tag="mm",
                                          name=f"mm{lno}_{mt}_{j}")
                    for g in range(KG):
                        nc.tensor.matmul(
                            ps[:, q, :],
                            lhsT=wq[:, 2 * g:2 * g + 2, mt * P:(mt + 1) * P],
                            rhs=rhs_of(g),
                            start=(g == 0), stop=(g == KG - 1),
                            perf_mode=DR,
                        )
                    act_relu(mt % 2 == 0, Hout[mt // 2][:, mt % 2, :],
                             ps[:, q, :], bp[:, mt:mt + 1])

            def emit_l1(j):
                emit_layer(j, 1, 2, 16, w1q, b1p, lambda g: PT[(g, j)][:])

            def emit_l2(j):
                emit_layer(j, 2, 8, 8, w2q, b2p, lambda g: H[(1, j)][g][:])

            def emit_l3(j):
                emit_layer(j, 3, 4, 4, w3q, b3p, lambda g: H[(2, j)][g][:])

            def emit_l4(j):
                jsl = slice(j * NB, (j + 1) * NB)
                H3 = H[(3, j)]
                ps4 = psum_l4.tile([1, NB], f32, tag="l4", name=f"l4_{j}")
                for kt in range(4):
                    nc.tensor.matmul(
                        ps4[:], lhsT=w4q[:, kt:kt + 1],
                        rhs=H3[kt // 2][:, kt % 2, :],
                        start=(kt == 0), stop=(kt == 3),
                    )
                nc.scalar.activation(ypre_sb[:, jsl], ps4[:], AF.Identity)
                nc.sync.dma_start(ydram[:, jsl], ypre_sb[:, jsl])

            # ---- software-pipelined emission ----
            emit_casts(0)
            emit_squares(0)
            fm0 = emit_fm_mm(0)
            emit_casts(1)
            emit_squares(1)
            emit_l1(0)
            emit_fm_tail(0, *fm0)
            fm1 = emit_fm_mm(1)
            emit_l2(0)
            emit_fm_tail(1, *fm1)
            emit_casts(2)
            emit_squares(2)
            emit_l3(0)
            emit_l4(0)
            emit_l1(1)
            fm2 = emit_fm_mm(2)
            emit_fm_tail(2, *fm2)
            emit_casts(3)
            emit_squares(3)
            emit_l2(1)
            emit_l3(1)
            emit_l4(1)
            emit_l1(2)
            fm3 = emit_fm_mm(3)
            emit_fm_tail(3, *fm3)

            # FM partial -> cross-core AllGather (hidden under the MLP)
            gp = const.tile([1, 1], f32, tag="gp")
            nc.vector.reduce_sum(out=gp[:], in_=gacc[:], axis=AX.X)
            sv128 = const.tile([P, 1], f32, tag="sv128")
            if use_cc:
                gin = dram.tile([1, 1], f32, tag="gin")
                gout = dram.tile([1, n_cores], f32, tag="gout",
                                 addr_space="Shared")
                nc.gpsimd.dma_start(gin[:], gp[:])
                nc.gpsimd.collective_compute(
                    "AllGather",
                    mybir.AluOpType.bypass,
                    replica_groups=[list(range(n_cores))],
                    ins=[gin.opt()],
                    outs=[gout.opt()],
                )
                gsb = const.tile([1, n_cores], f32, tag="gsb")
                nc.sync.dma_start(gsb[:], gout[:])
                gsum = const.tile([1, 1], f32, tag="gsum")
                nc.vector.reduce_sum(out=gsum[:], in_=gsb[:], axis=AX.X)
                # S = bias + b4 + 0.5 * sum(gparts)
                sv = const.tile([1, 1], f32, tag="sv")
                nc.scalar.activation(sv[:], gsum[:], AF.Identity,
                                     bias=bc_sb[0:1, 0:1], scale=0.5)
                nc.gpsimd.partition_broadcast(sv128[:], sv[:])

            emit_l2(2)
            emit_l3(2)
            emit_l4(2)
            emit_l1(3)
            emit_l2(3)
            emit_l3(3)
            emit_l4(3)

            # ---- tail: batch-major combine ----
            # linT[p, c] = sum_f fcv[p, f, c]
            linT = const.tile([P, NCH], f32, tag="linT")
            nc.vector.tensor_tensor(out=linT[:], in0=fcv[:, 0, :],
                                    in1=fcv[:, 1, :], op=ALU.add)
            lin2 = const.tile([P, NCH], f32, tag="lin2")
            nc.vector.tensor_tensor(out=lin2[:], in0=fcv[:, 2, :],
                                    in1=fcv[:, 3, :], op=ALU.add)
            nc.vector.tensor_tensor(out=linT[:], in0=linT[:], in1=lin2[:],
                                    op=ALU.add)
            # ypre [1, b_loc] -> batch-major [128, NCH] via a DRAM bounce
            # (direct SBUF->SBUF transposed DMA raced with the writers)
            ypreT = const.tile([P, NCH], f32, tag="ypreT")
            nc.sync.dma_start(
                ypreT[:], ydram.rearrange("o (c p) -> p (c o)", p=P))
            nc.vector.tensor_tensor(out=ypreT[:], in0=ypreT[:], in1=linT[:],
                                    op=ALU.add)
            if use_cc:
                ysb = const.tile([P, NCH], f32, tag="ysb")
                nc.scalar.activation(ysb[:], ypreT[:], AF.Sigmoid,
                                     bias=sv128[:])
                nc.sync.dma_start(y_d.rearrange("(c p) o -> p (c o)", p=P),
                                  ysb[:])
            else:
                nc.sync.dma_start(ylin_d, ypreT[:])
                nc.sync.dma_start(gpart_d, gp[:])

    nc.compile()
    return nc


def _build_b(b_loc, n_cores):
    """Fallback phase B (no-collective mode): y = sigmoid(ylin + S)."""
    import concourse.mybir as mybir
    import concourse.tile as tile
    from concourse import bacc

    f32 = mybir.dt.float32
    AF = mybir.ActivationFunctionType
    NCH = b_loc // P

    nc = bacc.Bacc(
        "TRN2",
        target_bir_lowering=False,
        debug=False,
        num_devices=n_cores,
    )
    yin_d = nc.dram_tensor("yin", [P, NCH], f32, kind="ExternalInput").ap()
    sv_d = nc.dram_tensor("sv", [P, 1], f32, kind="ExternalInput").ap()
    y_d = nc.dram_tensor("y", [b_loc, 1], f32, kind="ExternalOutput").ap()

    with tile.TileContext(nc) as tc:
        with tc.tile_pool(name="const", bufs=1) as const:
            yin = const.tile([P, NCH], f32, tag="yin")
            nc.sync.dma_start(yin[:], yin_d)
            sv = const.tile([P, 1], f32, tag="sv")
            nc.sync.dma_start(sv[:], sv_d)
            ysb = const.tile([P, NCH], f32, tag="ysb")
            nc.scalar.activation(ysb[:], yin[:], AF.Sigmoid, bias=sv[:])
            nc.sync.dma_start(y_d.rearrange("(c p) o -> p (c o)", p=P), ysb[:])

    nc.compile()
    return nc


def _get_program(phase, b_loc, n_cores, use_cc=True):
    key = (phase, b_loc, n_cores, use_cc)
    if key not in _build_cache:
        _build_cache[key] = (
            _build_main(b_loc, n_cores, use_cc) if phase == "A"
            else _build_b(b_loc, n_cores)
        )
    return _build_cache[key]


def _wrap_idx(lin_idx):
    """[n] int -> [128, n//16] int16 dma_gather index tile (16-wrap,
    replicated for the 8 Q7 cores)."""
    n = lin_idx.shape[0]
    wrap = lin_idx.astype(np.int16).reshape(n // 16, 16).T  # [16, n//16]
    return np.ascontiguousarray(np.tile(wrap, (8, 1)))


def _prep_shared(inputs):
    """Host-side table/weight prep shared by all cores."""
    bf = ml_dtypes.bfloat16
    f8 = ml_dtypes.float8_e4m3
    emb16 = np.asarray(inputs["emb_table"], np.float32).astype(bf)  # [T,128]

    sh = {}
    for f in range(F):
        # reference quirk: embedding lookup uses RAW (un-offset) ids into
        # the full table, so every field's table is the FIRST size_f rows
        sh[f"tab{f}"] = np.ascontiguousarray(emb16[:CAT_SIZES[f]])
    sh["fc"] = np.ascontiguousarray(np.asarray(inputs["fc"], np.float32))

    def dr_pack(w, kgroups):
        K, M = w.shape
        w = np.asarray(w, np.float32).reshape(kgroups, 2, P, M)
        return np.ascontiguousarray(
            w.transpose(2, 0, 1, 3).reshape(P, 2 * kgroups, M).astype(f8)
        )

    sh["w1q"] = dr_pack(np.asarray(inputs["W1"]), 2)
    sh["w2q"] = dr_pack(np.asarray(inputs["W2"]), 8)
    sh["w3q"] = dr_pack(np.asarray(inputs["W3"]), 4)
    sh["w4q"] = np.ascontiguousarray(
        np.asarray(inputs["W4"], np.float32).reshape(4, P).T.astype(f8)
    )
    for name, mt in (("b1", 16), ("b2", 8), ("b3", 4)):
        sh[f"{name}p"] = np.ascontiguousarray(
            np.asarray(inputs[name], np.float32).reshape(mt, P).T
        )
    bconst = (np.asarray(inputs["bias"], np.float32).reshape(-1)[0]
              + np.asarray(inputs["b4"], np.float32).reshape(-1)[0])
    sh["bconst"] = np.full((1, 1), bconst, dtype=np.float32)
    return sh


def _pack_ix(xs):
    """Per-core [b_loc, F] raw ids -> [128, NJ*F*NIX] int16, (j, f)-block
    order matching the kernel's ixsl()."""
    b_loc = xs.shape[0]
    NJ = b_loc // NB
    cols = []
    for j in range(NJ):
        for f in range(F):
            cols.append(_wrap_idx(xs[j * NB:(j + 1) * NB, f]))
    return np.ascontiguousarray(np.concatenate(cols, axis=1))


def kernel(**inputs) -> np.ndarray:
    from concourse.bass_utils import run_bass_kernel_spmd

    n_cores = N_CORES
    b_loc = B // n_cores
    NCH = b_loc // P
    cores = list(range(n_cores))
    trace = bool(int(os.environ.get("KERNEL_TRACE", "0")))
    use_cc = not bool(int(os.environ.get("KERNEL_NO_CC", "0")))

    x_int = np.asarray(inputs["x"], np.float32).astype(np.int32)  # [B, F]
    shared = _prep_shared(inputs)

    ncA = _get_program("A", b_loc, n_cores, use_cc)
    in_maps = []
    for c in range(n_cores):
        m = dict(shared)
        xs = x_int[c * b_loc:(c + 1) * b_loc]
        m["ix"] = _pack_ix(xs)
        # xig[p, f, c] = global fc id of batch row c*128+p
        m["xig"] = np.ascontiguousarray(
            (xs + OFFSETS_NP).reshape(NCH, P, F).transpose(1, 2, 0)
        )
        in_maps.append(m)
    resA = run_bass_kernel_spmd(ncA, in_maps, core_ids=cores, trace=trace)

    if use_cc:
        kernel._last_results = (resA,)
        kernel._last_exec_ns = resA.exec_time_ns
        kernel._last_exec_parts = (resA.exec_time_ns,)
        out = np.concatenate(
            [np.asarray(r["y"], np.float32).reshape(b_loc) for r in resA.results]
        )
        return out.reshape(B, 1).astype(np.float32)

    # ---- fallback: host-side reduction + tiny phase B ----
    g = np.float32(0.0)
    for r in resA.results:
        g = np.float32(g + np.float32(r["gpart"][0, 0]))
    S = np.float32(shared["bconst"][0, 0] + 0.5 * g)

    ncB = _get_program("B", b_loc, n_cores)
    sv = np.full((P, 1), S, dtype=np.float32)
    in_maps_b = [
        {"yin": np.asarray(resA.results[c]["ylin"], np.float32), "sv": sv}
        for c in range(n_cores)
    ]
    resB = run_bass_kernel_spmd(ncB, in_maps_b, core_ids=cores, trace=trace)

    kernel._last_results = (resA, resB)
    a_ns, b_ns = resA.exec_time_ns, resB.exec_time_ns
    kernel._last_exec_ns = (
        (a_ns or 0) + (b_ns or 0) if (a_ns is not None or b_ns is not None)
        else None
    )
    kernel._last_exec_parts = (a_ns, b_ns)
    out = np.concatenate([r["y"] for r in resB.results], axis=0)
    return out.astype(np.float32)
